# revision 22
# baseline (speedup 1.0000x reference)
"""LDEPool1d Trainium2 Bass kernel (v6).

Reference computation (B=16, T=800, D=256, K=64):
    delta = x[:,:,None,:] - mu[None,None,:,:]          # (B,T,K,D)
    dist  = sum(delta*delta, -1)                       # (B,T,K)
    llk   = -(prec*prec) * dist
    r     = softmax(llk, axis=-1)                      # over K
    r     = r / (sum(r, axis=1) + 1e-9)                # over T
    pool  = einsum('btk,btkd->bkd', r, delta)          # (B,K,D)
    out   = pool.reshape(B, K*D)

Kernel algebra (per batch b, p2 = prec^2):
    llk[t,k] = 2*p2*mu[k]·x[t] + nb[k] + const(t); nb = -p2*||mu_k||^2;
    const(t) cancels in the softmax over k.
    m_t = max_k llk; e = exp(llk - m_t); r~ = (2^15/Z_t) e
    pool[k,d] = sum_t r~[t,k]*x[t,d]  (ones col appended to x gives
    S_k = sum_t r~);  out = pool*Sr - mu*(S*Sr),  Sr = 1/(S+2^15*eps)

v6 structure:
  - Host prep: x -> fp16 with ones column (halves DMA, kills the
    on-device convert pass); mu_sT fp16, nb hi/lo fp16 rows and the
    128x128 fp16 identity are all shipped as constants (no on-device
    mu-prep chain, no make_identity).
  - mm1 computes llk directly in [t, K] layout: per t-chunk,
    lhsT = xT slice (weights), rhs = mu_sT half; the nb bias is seeded
    per (batch, group) by a wide matmul that opens the accumulation.
    No llk evac / transpose-back.
  - xT via PE transposes (DMA XBAR transpose serializes the whole DMA
    stream on this stack - measured, not worth it).
  - 4 streams (2 batches x 2 t-groups) software-pipelined; softmax as
    wide per-group ops spread over DVE/ACT/Pool; 2^15 rescale fused
    into the r16 multiply (scalar_tensor_tensor).

Sharding: data-parallel over B across 8 cores (2 batches/core), mu/prec
replicated. No collectives.
"""

import sys

if "/opt/trn_rl_repo" not in sys.path:
    sys.path.insert(0, "/opt/trn_rl_repo")

import numpy as np

B, T, D, K = 16, 800, 256, 64
N_CORES = 8
B_LOC = B // N_CORES
EPS = 1e-9
DP = D + 2  # x row: 256 data + ones col + pad
NCH = 7
CHUNKS = [(t0, min(128, T - t0)) for t0 in range(0, T, 128)]
# groups: (first chunk, n chunks, t offset, t width)
GROUPS = [(0, 4, 0, 512), (4, 3, 512, 288)]
STREAMS = [(b, g) for b in range(B_LOC) for g in range(2)]


def _emit(tc, x_d, c16a_d, muf_d, out_d):
    import concourse.bass as bass
    from concourse import mybir
    from contextlib import ExitStack

    f32 = mybir.dt.float32
    f16 = mybir.dt.float16
    nc = tc.nc
    AF = mybir.ActivationFunctionType
    ALU = mybir.AluOpType

    ctx = ExitStack()
    const = ctx.enter_context(tc.tile_pool(name="const", bufs=1))
    xp = ctx.enter_context(tc.tile_pool(name="x16", bufs=1))
    xtp = ctx.enter_context(tc.tile_pool(name="xt", bufs=1))
    smp = ctx.enter_context(tc.tile_pool(name="sm", bufs=1))
    epip = ctx.enter_context(tc.tile_pool(name="epi", bufs=2))
    ps_xt = ctx.enter_context(tc.tile_pool(name="ps_xt", bufs=2, space="PSUM"))
    ps_la = ctx.enter_context(tc.tile_pool(name="ps_la", bufs=2, space="PSUM"))
    ps_p = ctx.enter_context(tc.tile_pool(name="ps_p", bufs=2, space="PSUM"))

    # packed fp16 consts: 0:128 eye | 128:256 musT | 256:320 nb2 | 320:448 ones
    c16a = const.tile([128, 448], f16)
    muf = const.tile([K, D], f32)
    x16 = xp.tile([128, B_LOC, NCH, DP], f16, tag="x16")
    xT = xtp.tile([128, B_LOC, 2, T], f16, tag="xT")
    nm = smp.tile([128, B_LOC, NCH], f32, tag="nm")
    s = smp.tile([128, B_LOC, NCH, K], f32, tag="s")
    e = smp.tile([128, B_LOC, NCH, K], f32, tag="e")
    z = smp.tile([128, B_LOC, NCH], f32, tag="z")
    zr = smp.tile([128, B_LOC, NCH], f32, tag="zr")
    r16 = smp.tile([128, B_LOC, NCH, K], f16, tag="r16")

    ident = c16a[:, 0:128]
    musT = [c16a[:, 128 + h * K : 128 + (h + 1) * K] for h in range(2)]

    # ---------------- DMA issue ----------------
    # scalar queue: tiny consts only (it must be free for ACT compute);
    # sync queue: x16 then (later) the outputs.  x16 is loaded p-major
    # ("(p c) d"): each partition's 6 chunk-rows are contiguous in HBM
    # (3KB packets).  This permutes t within chunks, which is harmless:
    # every downstream op is per-(partition, chunk)-row consistent and
    # the final pool sums over t.
    nc.scalar.dma_start(out=c16a, in_=c16a_d)
    for b in range(B_LOC):
        xv = x_d[b, 0:768, :].rearrange("(p c) d -> p c d", c=6)
        nc.sync.dma_start(out=x16[:, b, 0:2, :], in_=xv[:, 0:2, :])
        nc.sync.dma_start(out=x16[:, b, 2:4, :], in_=xv[:, 2:4, :])
        nc.scalar.dma_start(out=x16[:, b, 4:6, :], in_=xv[:, 4:6, :])
        nc.scalar.dma_start(out=x16[0:32, b, 6, :], in_=x_d[b, 768:800, :])
    nc.scalar.dma_start(out=muf, in_=muf_d)

    # ---------------- stages ----------------
    la = [
        ps_la.tile([128, NCH, K], f32, tag="la", name=f"la{b}")
        for b in range(B_LOC)
    ]
    xt_ps = {}
    pool_ps = {}

    def bias(b):
        # One whole-tile bias matmul opens the bank's accumulation: a
        # start=True write marks the ENTIRE psum bank pending-zero, so
        # there must be exactly one start per bank, covering everything
        # the chunk matmuls later accumulate into.
        nb_bc = c16a[0:2, 256:320].unsqueeze(1).broadcast_to((2, NCH, K))
        nc.tensor.matmul(
            la[b],
            lhsT=c16a[0:2, 320:448],
            rhs=nb_bc,
            start=True, stop=False, skip_group_check=True,
        )

    def tx(b, g):
        c0, ncc, toff, _ = GROUPS[g]
        p = ps_xt.tile([128, 2, 512], f16, tag="xt", name=f"xt{b}{g}")
        xt_ps[(b, g)] = p
        for h in range(2):
            for c in range(c0, c0 + ncc):
                t0, tcn = CHUNKS[c]
                nc.tensor.transpose(
                    p[:, h, t0 - toff : t0 - toff + tcn],
                    x16[0:tcn, b, c, h * 128 : (h + 1) * 128],
                    ident[0:tcn, 0:tcn],
                )

    def evac(b, g):
        # one wide copy per stream, alternating ACT/DVE across streams
        _, _, toff, tw = GROUPS[g]
        p = xt_ps[(b, g)]
        dst = xT[:, b, :, toff : toff + tw]
        if (2 * b + g) % 2 == 0:
            nc.vector.tensor_copy(dst, p[:, :, 0:tw])
        else:
            nc.scalar.copy(dst, p[:, :, 0:tw])

    def mm1(b, g):
        c0, ncc, _, _ = GROUPS[g]
        for c in range(c0, c0 + ncc):
            t0, tcn = CHUNKS[c]
            for h in range(2):
                nc.tensor.matmul(
                    la[b][0:tcn, c, :],
                    lhsT=xT[:, b, h, t0 : t0 + tcn],
                    rhs=musT[h],
                    start=False,
                    stop=(c == c0 + ncc - 1 and h == 1),
                    skip_group_check=True,
                )

    def smf(b, g):
        # softmax front: row max (negated), subtract, exp
        c0, ncc, _, _ = GROUPS[g]
        cs = slice(c0, c0 + ncc)
        nc.vector.tensor_reduce(
            out=nm[:, b, cs], in_=la[b][:, cs, :],
            axis=mybir.AxisListType.X, op=ALU.max, negate=True,
        )
        nm_bc = nm[:, b, cs].unsqueeze(2).broadcast_to((128, ncc, K))
        nc.vector.tensor_add(s[:, b, cs, :], la[b][:, cs, :], nm_bc)
        nc.scalar.activation(out=e[:, b, cs, :], in_=s[:, b, cs, :], func=AF.Exp)

    def smb(b, g):
        # softmax back: Z, 2^15/Z, r~ = e * (2^15/Z)
        c0, ncc, _, _ = GROUPS[g]
        cs = slice(c0, c0 + ncc)
        nc.vector.tensor_reduce(
            out=z[:, b, cs], in_=e[:, b, cs, :],
            axis=mybir.AxisListType.X, op=ALU.add,
        )
        nc.vector.reciprocal(zr[:, b, cs], z[:, b, cs])
        # scale r~ by 2^15 before the fp16 cast so near-dead components'
        # tiny weights stay above the fp16 flush threshold; the epilogue
        # divides by (S + 2^15*eps), which cancels the scale exactly.
        nc.vector.tensor_scalar_mul(zr[:, b, cs], zr[:, b, cs], 32768.0)
        zr_bc = zr[:, b, cs].unsqueeze(2).broadcast_to((128, ncc, K))
        nc.gpsimd.tensor_mul(r16[:, b, cs, :], e[:, b, cs, :], zr_bc)

    def mm2(b, g):
        c0, ncc, _, _ = GROUPS[g]
        if g == 0:
            pool_ps[b] = ps_p.tile([K, D + 1], f32, tag="pool", name=f"pool{b}")
        for c in range(c0, c0 + ncc):
            t0, tcn = CHUNKS[c]
            nc.tensor.matmul(
                pool_ps[b],
                lhsT=r16[0:tcn, b, c, :],
                rhs=x16[0:tcn, b, c, 0 : D + 1],
                start=(c == 0),
                stop=(c == NCH - 1),
            )

    def epilogue(b):
        # po = (pool - mu*S) * Sr,  Sr = 1/(S + 2^15*eps); exactly
        # pool*Sr - mu*(S*Sr) but with a 3-hop chain instead of 5.
        pp = pool_ps[b]
        negS = epip.tile([K, 1], f32, tag="negS")
        se = epip.tile([K, 1], f32, tag="se")
        sr = epip.tile([K, 1], f32, tag="sr")
        q = epip.tile([K, D], f32, tag="q")
        po = epip.tile([K, D], f32, tag="po")
        S = pp[:, D : D + 1]
        nc.vector.tensor_scalar_mul(negS, S, -1.0)
        nc.vector.tensor_scalar_add(se, S, EPS * 32768.0)
        nc.vector.reciprocal(sr, se)
        nc.vector.scalar_tensor_tensor(
            out=q, in0=muf, scalar=negS, in1=pp[:, 0:D],
            op0=ALU.mult, op1=ALU.add,
        )
        nc.vector.tensor_scalar_mul(po, q, sr)
        nc.sync.dma_start(
            out=out_d[b, :].rearrange("(k d) -> k d", k=K), in_=po
        )

    # ---------------- software-pipelined emission ----------------
    S0, S1, S2, S3 = STREAMS
    tx(*S0)
    tx(*S1)
    bias(0)
    evac(*S0)
    mm1(*S0)
    tx(*S2)
    bias(1)
    evac(*S1)
    mm1(*S1)
    smf(*S0)
    tx(*S3)
    evac(*S2)
    mm1(*S2)
    smf(*S1)
    smb(*S0)
    evac(*S3)
    mm1(*S3)
    smf(*S2)
    smb(*S1)
    mm2(*S0)
    mm2(*S1)
    smf(*S3)
    smb(*S2)
    epilogue(0)
    mm2(*S2)
    smb(*S3)
    mm2(*S3)
    epilogue(1)
    ctx.close()


_NC = None


def _get_nc():
    global _NC
    if _NC is None:
        import concourse.bacc as bacc
        import concourse.tile as tile
        from concourse import mybir

        f32 = mybir.dt.float32
        f16 = mybir.dt.float16
        nc = bacc.Bacc(
            "TRN2", target_bir_lowering=False, debug=False, num_devices=N_CORES
        )
        x_d = nc.dram_tensor(
            "x16", [B_LOC, T, DP], f16, kind="ExternalInput"
        ).ap()
        c16a_d = nc.dram_tensor(
            "c16a", [128, 448], f16, kind="ExternalInput"
        ).ap()
        muf_d = nc.dram_tensor("muf", [K, D], f32, kind="ExternalInput").ap()
        out_d = nc.dram_tensor(
            "out", [B_LOC, K * D], f32, kind="ExternalOutput"
        ).ap()
        with tile.TileContext(nc) as tc:
            _emit(tc, x_d, c16a_d, muf_d, out_d)
        nc.compile()
        _NC = nc
    return _NC


def _host_prep(x, mu, prec):
    """Host-side input prep: fp16 x with ones col, const tensors."""
    x = np.asarray(x, dtype=np.float32)
    mu64 = np.asarray(mu, dtype=np.float64)
    p2 = np.asarray(prec, dtype=np.float64) ** 2

    x16 = np.zeros((B, T, DP), dtype=np.float16)
    x16[:, :, 0:D] = x.astype(np.float16)
    x16[:, :, D] = 1.0

    mu_s16 = (2.0 * p2[:, None] * mu64).astype(np.float16)  # [K, D]
    c16a = np.zeros((128, 448), dtype=np.float16)
    c16a[:, 0:128] = np.eye(128, dtype=np.float16)
    c16a[:, 128:192] = mu_s16.T[0:128, :]
    c16a[:, 192:256] = mu_s16.T[128:256, :]

    nb = -(p2 * (mu64 * mu64).sum(-1))  # [K], exact in f64
    nb_hi = nb.astype(np.float16)
    nb_lo = (nb - nb_hi.astype(np.float64)).astype(np.float16)
    c16a[0, 256:320] = nb_hi
    c16a[1, 256:320] = nb_lo
    c16a[0:2, 320:448] = 1.0

    muf = np.ascontiguousarray(mu64.astype(np.float32))
    return x16, c16a, muf


def kernel(x, mu, prec, **_ignored):
    from concourse.bass_utils import run_bass_kernel_spmd

    x16, c16a, muf = _host_prep(x, mu, prec)
    nc = _get_nc()
    in_maps = [
        {
            "x16": np.ascontiguousarray(x16[c * B_LOC : (c + 1) * B_LOC]),
            "c16a": c16a,
            "muf": muf,
        }
        for c in range(N_CORES)
    ]
    res = run_bass_kernel_spmd(nc, in_maps, list(range(N_CORES)))
    return np.concatenate(
        [res.results[c]["out"] for c in range(N_CORES)], axis=0
    ).astype(np.float32)


# revision 23
# speedup vs baseline: 1.0810x; 1.0810x over previous
"""LDEPool1d Trainium2 Bass kernel (v6).

Reference computation (B=16, T=800, D=256, K=64):
    delta = x[:,:,None,:] - mu[None,None,:,:]          # (B,T,K,D)
    dist  = sum(delta*delta, -1)                       # (B,T,K)
    llk   = -(prec*prec) * dist
    r     = softmax(llk, axis=-1)                      # over K
    r     = r / (sum(r, axis=1) + 1e-9)                # over T
    pool  = einsum('btk,btkd->bkd', r, delta)          # (B,K,D)
    out   = pool.reshape(B, K*D)

Kernel algebra (per batch b, p2 = prec^2):
    llk[t,k] = 2*p2*mu[k]·x[t] + nb[k] + const(t); nb = -p2*||mu_k||^2;
    const(t) cancels in the softmax over k.
    m_t = max_k llk; e = exp(llk - m_t); r~ = (2^15/Z_t) e
    pool[k,d] = sum_t r~[t,k]*x[t,d]  (ones col appended to x gives
    S_k = sum_t r~);  out = pool*Sr - mu*(S*Sr),  Sr = 1/(S+2^15*eps)

v6 structure:
  - Host prep: x -> fp16 with ones column (halves DMA, kills the
    on-device convert pass); mu_sT fp16, nb hi/lo fp16 rows and the
    128x128 fp16 identity are all shipped as constants (no on-device
    mu-prep chain, no make_identity).
  - mm1 computes llk directly in [t, K] layout: per t-chunk,
    lhsT = xT slice (weights), rhs = mu_sT half; the nb bias is seeded
    per (batch, group) by a wide matmul that opens the accumulation.
    No llk evac / transpose-back.
  - xT via PE transposes (DMA XBAR transpose serializes the whole DMA
    stream on this stack - measured, not worth it).
  - 4 streams (2 batches x 2 t-groups) software-pipelined; softmax as
    wide per-group ops spread over DVE/ACT/Pool; 2^15 rescale fused
    into the r16 multiply (scalar_tensor_tensor).

Sharding: data-parallel over B across 8 cores (2 batches/core), mu/prec
replicated. No collectives.
"""

import sys

if "/opt/trn_rl_repo" not in sys.path:
    sys.path.insert(0, "/opt/trn_rl_repo")

import numpy as np

B, T, D, K = 16, 800, 256, 64
N_CORES = 8
B_LOC = B // N_CORES
EPS = 1e-9
DP = D + 2  # x row: 256 data + ones col + pad
NCH = 7
CHUNKS = [(t0, min(128, T - t0)) for t0 in range(0, T, 128)]
# groups: (first chunk, n chunks, t offset, t width)
GROUPS = [(0, 4, 0, 512), (4, 3, 512, 288)]
STREAMS = [(b, g) for b in range(B_LOC) for g in range(2)]


def _emit(tc, x_d, c16a_d, muf_d, out_d):
    import concourse.bass as bass
    from concourse import mybir
    from contextlib import ExitStack

    f32 = mybir.dt.float32
    f16 = mybir.dt.float16
    nc = tc.nc
    AF = mybir.ActivationFunctionType
    ALU = mybir.AluOpType

    ctx = ExitStack()
    const = ctx.enter_context(tc.tile_pool(name="const", bufs=1))
    xp = ctx.enter_context(tc.tile_pool(name="x16", bufs=1))
    xtp = ctx.enter_context(tc.tile_pool(name="xt", bufs=1))
    smp = ctx.enter_context(tc.tile_pool(name="sm", bufs=1))
    epip = ctx.enter_context(tc.tile_pool(name="epi", bufs=2))
    ps_xt = ctx.enter_context(tc.tile_pool(name="ps_xt", bufs=2, space="PSUM"))
    ps_la = ctx.enter_context(tc.tile_pool(name="ps_la", bufs=2, space="PSUM"))
    ps_p = ctx.enter_context(tc.tile_pool(name="ps_p", bufs=2, space="PSUM"))

    # packed fp16 consts: 0:128 eye | 128:256 musT | 256:320 nb2 | 320:448 ones
    c16a = const.tile([128, 448], f16)
    muf = const.tile([K, D], f32)
    x16 = xp.tile([128, B_LOC, NCH, DP], f16, tag="x16")
    xT = xtp.tile([128, B_LOC, 2, T], f16, tag="xT")
    nm = smp.tile([128, B_LOC, NCH], f32, tag="nm")
    s = smp.tile([128, B_LOC, NCH, K], f32, tag="s")
    e = smp.tile([128, B_LOC, NCH, K], f32, tag="e")
    z = smp.tile([128, B_LOC, NCH], f32, tag="z")
    zr = smp.tile([128, B_LOC, NCH], f32, tag="zr")
    r16 = smp.tile([128, B_LOC, NCH, K], f16, tag="r16")

    ident = c16a[:, 0:128]
    musT = [c16a[:, 128 + h * K : 128 + (h + 1) * K] for h in range(2)]

    # ---------------- DMA issue ----------------
    # scalar queue: tiny consts only (it must be free for ACT compute);
    # sync queue: x16 then (later) the outputs.  x16 is loaded p-major
    # ("(p c) d"): each partition's 6 chunk-rows are contiguous in HBM
    # (3KB packets).  This permutes t within chunks, which is harmless:
    # every downstream op is per-(partition, chunk)-row consistent and
    # the final pool sums over t.
    nc.sync.dma_start(out=c16a, in_=c16a_d)
    for b in range(B_LOC):
        xv = x_d[b, 0:768, :].rearrange("(p c) d -> p c d", c=6)
        nc.sync.dma_start(out=x16[:, b, 0:2, :], in_=xv[:, 0:2, :])
        nc.sync.dma_start(out=x16[:, b, 2:4, :], in_=xv[:, 2:4, :])
        nc.scalar.dma_start(out=x16[:, b, 4:6, :], in_=xv[:, 4:6, :])
        nc.scalar.dma_start(out=x16[0:32, b, 6, :], in_=x_d[b, 768:800, :])
    nc.scalar.dma_start(out=muf, in_=muf_d)

    # ---------------- stages ----------------
    la = [
        ps_la.tile([128, NCH, K], f32, tag="la", name=f"la{b}")
        for b in range(B_LOC)
    ]
    xt_ps = {}
    pool_ps = {}

    def bias(b):
        # One whole-tile bias matmul opens the bank's accumulation: a
        # start=True write marks the ENTIRE psum bank pending-zero, so
        # there must be exactly one start per bank, covering everything
        # the chunk matmuls later accumulate into.
        nb_bc = c16a[0:2, 256:320].unsqueeze(1).broadcast_to((2, NCH, K))
        nc.tensor.matmul(
            la[b],
            lhsT=c16a[0:2, 320:448],
            rhs=nb_bc,
            start=True, stop=False, skip_group_check=True,
        )

    def tx(b, g):
        c0, ncc, toff, _ = GROUPS[g]
        p = ps_xt.tile([128, 2, 512], f16, tag="xt", name=f"xt{b}{g}")
        xt_ps[(b, g)] = p
        for h in range(2):
            for c in range(c0, c0 + ncc):
                t0, tcn = CHUNKS[c]
                nc.tensor.transpose(
                    p[:, h, t0 - toff : t0 - toff + tcn],
                    x16[0:tcn, b, c, h * 128 : (h + 1) * 128],
                    ident[0:tcn, 0:tcn],
                )

    def evac(b, g):
        # one wide copy per stream, alternating ACT/DVE across streams
        _, _, toff, tw = GROUPS[g]
        p = xt_ps[(b, g)]
        dst = xT[:, b, :, toff : toff + tw]
        if (2 * b + g) % 2 == 0:
            nc.vector.tensor_copy(dst, p[:, :, 0:tw])
        else:
            nc.scalar.copy(dst, p[:, :, 0:tw])

    def mm1(b, g):
        c0, ncc, _, _ = GROUPS[g]
        for c in range(c0, c0 + ncc):
            t0, tcn = CHUNKS[c]
            for h in range(2):
                nc.tensor.matmul(
                    la[b][0:tcn, c, :],
                    lhsT=xT[:, b, h, t0 : t0 + tcn],
                    rhs=musT[h],
                    start=False,
                    stop=(c == c0 + ncc - 1 and h == 1),
                    skip_group_check=True,
                )

    def smf(b, g):
        # softmax front: row max (negated), subtract, exp
        c0, ncc, _, _ = GROUPS[g]
        cs = slice(c0, c0 + ncc)
        nc.vector.tensor_reduce(
            out=nm[:, b, cs], in_=la[b][:, cs, :],
            axis=mybir.AxisListType.X, op=ALU.max, negate=True,
        )
        nm_bc = nm[:, b, cs].unsqueeze(2).broadcast_to((128, ncc, K))
        nc.vector.tensor_add(s[:, b, cs, :], la[b][:, cs, :], nm_bc)
        nc.scalar.activation(out=e[:, b, cs, :], in_=s[:, b, cs, :], func=AF.Exp)

    def smb(b, g):
        # softmax back: Z, 2^15/Z, r~ = e * (2^15/Z)
        c0, ncc, _, _ = GROUPS[g]
        cs = slice(c0, c0 + ncc)
        nc.vector.tensor_reduce(
            out=z[:, b, cs], in_=e[:, b, cs, :],
            axis=mybir.AxisListType.X, op=ALU.add,
        )
        nc.vector.reciprocal(zr[:, b, cs], z[:, b, cs])
        # scale r~ by 2^15 before the fp16 cast so near-dead components'
        # tiny weights stay above the fp16 flush threshold; the epilogue
        # divides by (S + 2^15*eps), which cancels the scale exactly.
        nc.vector.tensor_scalar_mul(zr[:, b, cs], zr[:, b, cs], 32768.0)
        zr_bc = zr[:, b, cs].unsqueeze(2).broadcast_to((128, ncc, K))
        nc.gpsimd.tensor_mul(r16[:, b, cs, :], e[:, b, cs, :], zr_bc)

    def mm2(b, g):
        c0, ncc, _, _ = GROUPS[g]
        if g == 0:
            pool_ps[b] = ps_p.tile([K, D + 1], f32, tag="pool", name=f"pool{b}")
        for c in range(c0, c0 + ncc):
            t0, tcn = CHUNKS[c]
            nc.tensor.matmul(
                pool_ps[b],
                lhsT=r16[0:tcn, b, c, :],
                rhs=x16[0:tcn, b, c, 0 : D + 1],
                start=(c == 0),
                stop=(c == NCH - 1),
            )

    def epilogue(b):
        # po = (pool - mu*S) * Sr,  Sr = 1/(S + 2^15*eps); exactly
        # pool*Sr - mu*(S*Sr) but with a 3-hop chain instead of 5.
        pp = pool_ps[b]
        negS = epip.tile([K, 1], f32, tag="negS")
        se = epip.tile([K, 1], f32, tag="se")
        sr = epip.tile([K, 1], f32, tag="sr")
        q = epip.tile([K, D], f32, tag="q")
        po = epip.tile([K, D], f32, tag="po")
        S = pp[:, D : D + 1]
        nc.vector.tensor_scalar_mul(negS, S, -1.0)
        nc.vector.tensor_scalar_add(se, S, EPS * 32768.0)
        nc.vector.reciprocal(sr, se)
        nc.vector.scalar_tensor_tensor(
            out=q, in0=muf, scalar=negS, in1=pp[:, 0:D],
            op0=ALU.mult, op1=ALU.add,
        )
        nc.vector.tensor_scalar_mul(po, q, sr)
        nc.sync.dma_start(
            out=out_d[b, :].rearrange("(k d) -> k d", k=K), in_=po
        )

    # ---------------- software-pipelined emission ----------------
    S0, S1, S2, S3 = STREAMS
    tx(*S0)
    tx(*S1)
    bias(0)
    evac(*S0)
    mm1(*S0)
    tx(*S2)
    bias(1)
    evac(*S1)
    mm1(*S1)
    smf(*S0)
    tx(*S3)
    evac(*S2)
    mm1(*S2)
    smf(*S1)
    smb(*S0)
    evac(*S3)
    mm1(*S3)
    smf(*S2)
    smb(*S1)
    mm2(*S0)
    mm2(*S1)
    smf(*S3)
    smb(*S2)
    epilogue(0)
    mm2(*S2)
    smb(*S3)
    mm2(*S3)
    epilogue(1)
    ctx.close()


_NC = None


def _get_nc():
    global _NC
    if _NC is None:
        import concourse.bacc as bacc
        import concourse.tile as tile
        from concourse import mybir

        f32 = mybir.dt.float32
        f16 = mybir.dt.float16
        nc = bacc.Bacc(
            "TRN2", target_bir_lowering=False, debug=False, num_devices=N_CORES
        )
        x_d = nc.dram_tensor(
            "x16", [B_LOC, T, DP], f16, kind="ExternalInput"
        ).ap()
        c16a_d = nc.dram_tensor(
            "c16a", [128, 448], f16, kind="ExternalInput"
        ).ap()
        muf_d = nc.dram_tensor("muf", [K, D], f32, kind="ExternalInput").ap()
        out_d = nc.dram_tensor(
            "out", [B_LOC, K * D], f32, kind="ExternalOutput"
        ).ap()
        with tile.TileContext(nc) as tc:
            _emit(tc, x_d, c16a_d, muf_d, out_d)
        nc.compile()
        _NC = nc
    return _NC


def _host_prep(x, mu, prec):
    """Host-side input prep: fp16 x with ones col, const tensors."""
    x = np.asarray(x, dtype=np.float32)
    mu64 = np.asarray(mu, dtype=np.float64)
    p2 = np.asarray(prec, dtype=np.float64) ** 2

    x16 = np.zeros((B, T, DP), dtype=np.float16)
    x16[:, :, 0:D] = x.astype(np.float16)
    x16[:, :, D] = 1.0

    mu_s16 = (2.0 * p2[:, None] * mu64).astype(np.float16)  # [K, D]
    c16a = np.zeros((128, 448), dtype=np.float16)
    c16a[:, 0:128] = np.eye(128, dtype=np.float16)
    c16a[:, 128:192] = mu_s16.T[0:128, :]
    c16a[:, 192:256] = mu_s16.T[128:256, :]

    nb = -(p2 * (mu64 * mu64).sum(-1))  # [K], exact in f64
    nb_hi = nb.astype(np.float16)
    nb_lo = (nb - nb_hi.astype(np.float64)).astype(np.float16)
    c16a[0, 256:320] = nb_hi
    c16a[1, 256:320] = nb_lo
    c16a[0:2, 320:448] = 1.0

    muf = np.ascontiguousarray(mu64.astype(np.float32))
    return x16, c16a, muf


def kernel(x, mu, prec, **_ignored):
    from concourse.bass_utils import run_bass_kernel_spmd

    x16, c16a, muf = _host_prep(x, mu, prec)
    nc = _get_nc()
    in_maps = [
        {
            "x16": np.ascontiguousarray(x16[c * B_LOC : (c + 1) * B_LOC]),
            "c16a": c16a,
            "muf": muf,
        }
        for c in range(N_CORES)
    ]
    res = run_bass_kernel_spmd(nc, in_maps, list(range(N_CORES)))
    return np.concatenate(
        [res.results[c]["out"] for c in range(N_CORES)], axis=0
    ).astype(np.float32)


# revision 24
# speedup vs baseline: 1.0950x; 1.0129x over previous
"""LDEPool1d Trainium2 Bass kernel (v6).

Reference computation (B=16, T=800, D=256, K=64):
    delta = x[:,:,None,:] - mu[None,None,:,:]          # (B,T,K,D)
    dist  = sum(delta*delta, -1)                       # (B,T,K)
    llk   = -(prec*prec) * dist
    r     = softmax(llk, axis=-1)                      # over K
    r     = r / (sum(r, axis=1) + 1e-9)                # over T
    pool  = einsum('btk,btkd->bkd', r, delta)          # (B,K,D)
    out   = pool.reshape(B, K*D)

Kernel algebra (per batch b, p2 = prec^2):
    llk[t,k] = 2*p2*mu[k]·x[t] + nb[k] + const(t); nb = -p2*||mu_k||^2;
    const(t) cancels in the softmax over k.
    m_t = max_k llk; e = exp(llk - m_t); r~ = (2^15/Z_t) e
    pool[k,d] = sum_t r~[t,k]*x[t,d]  (ones col appended to x gives
    S_k = sum_t r~);  out = pool*Sr - mu*(S*Sr),  Sr = 1/(S+2^15*eps)

v6 structure:
  - Host prep: x -> fp16 with ones column (halves DMA, kills the
    on-device convert pass); mu_sT fp16, nb hi/lo fp16 rows and the
    128x128 fp16 identity are all shipped as constants (no on-device
    mu-prep chain, no make_identity).
  - mm1 computes llk directly in [t, K] layout: per t-chunk,
    lhsT = xT slice (weights), rhs = mu_sT half; the nb bias is seeded
    per (batch, group) by a wide matmul that opens the accumulation.
    No llk evac / transpose-back.
  - xT via PE transposes (DMA XBAR transpose serializes the whole DMA
    stream on this stack - measured, not worth it).
  - 4 streams (2 batches x 2 t-groups) software-pipelined; softmax as
    wide per-group ops spread over DVE/ACT/Pool; 2^15 rescale fused
    into the r16 multiply (scalar_tensor_tensor).

Sharding: data-parallel over B across 8 cores (2 batches/core), mu/prec
replicated. No collectives.
"""

import sys

if "/opt/trn_rl_repo" not in sys.path:
    sys.path.insert(0, "/opt/trn_rl_repo")

import numpy as np

B, T, D, K = 16, 800, 256, 64
N_CORES = 8
B_LOC = B // N_CORES
EPS = 1e-9
DP = D + 2  # x row: 256 data + ones col + pad
NCH = 7
CHUNKS = [(t0, min(128, T - t0)) for t0 in range(0, T, 128)]
# groups: (first chunk, n chunks, t offset, t width)
GROUPS = [(0, 4, 0, 512), (4, 3, 512, 288)]
STREAMS = [(b, g) for b in range(B_LOC) for g in range(2)]


def _emit(tc, x_d, c16a_d, nbo_d, muf_d, out_d):
    import concourse.bass as bass
    from concourse import mybir
    from contextlib import ExitStack

    f32 = mybir.dt.float32
    f16 = mybir.dt.float16
    nc = tc.nc
    AF = mybir.ActivationFunctionType
    ALU = mybir.AluOpType

    ctx = ExitStack()
    const = ctx.enter_context(tc.tile_pool(name="const", bufs=1))
    xp = ctx.enter_context(tc.tile_pool(name="x16", bufs=1))
    xtp = ctx.enter_context(tc.tile_pool(name="xt", bufs=1))
    smp = ctx.enter_context(tc.tile_pool(name="sm", bufs=1))
    epip = ctx.enter_context(tc.tile_pool(name="epi", bufs=2))
    ps_xt = ctx.enter_context(tc.tile_pool(name="ps_xt", bufs=2, space="PSUM"))
    ps_la = ctx.enter_context(tc.tile_pool(name="ps_la", bufs=2, space="PSUM"))
    ps_p = ctx.enter_context(tc.tile_pool(name="ps_p", bufs=2, space="PSUM"))

    # packed fp16 consts: 0:128 eye | 128:256 musT
    c16a = const.tile([128, 256], f16)
    nbo = const.tile([2, 192], f16)  # 0:64 nb hi/lo rows, 64:192 ones
    muf = const.tile([K, D], f32)
    x16 = xp.tile([128, B_LOC, NCH, DP], f16, tag="x16")
    xT = xtp.tile([128, B_LOC, 2, T], f16, tag="xT")
    nm = smp.tile([128, B_LOC, NCH], f32, tag="nm")
    s = smp.tile([128, B_LOC, NCH, K], f32, tag="s")
    e = smp.tile([128, B_LOC, NCH, K], f32, tag="e")
    z = smp.tile([128, B_LOC, NCH], f32, tag="z")
    zr = smp.tile([128, B_LOC, NCH], f32, tag="zr")
    r16 = smp.tile([128, B_LOC, NCH, K], f16, tag="r16")

    ident = c16a[:, 0:128]
    musT = [c16a[:, 128 + h * K : 128 + (h + 1) * K] for h in range(2)]

    # ---------------- DMA issue ----------------
    # scalar queue: tiny consts only (it must be free for ACT compute);
    # sync queue: x16 then (later) the outputs.  x16 is loaded p-major
    # ("(p c) d"): each partition's 6 chunk-rows are contiguous in HBM
    # (3KB packets).  This permutes t within chunks, which is harmless:
    # every downstream op is per-(partition, chunk)-row consistent and
    # the final pool sums over t.
    nc.sync.dma_start(out=c16a, in_=c16a_d)
    nc.scalar.dma_start(out=nbo, in_=nbo_d)
    for b in range(B_LOC):
        xv = x_d[b, 0:768, :].rearrange("(p c) d -> p c d", c=6)
        nc.sync.dma_start(out=x16[:, b, 0:2, :], in_=xv[:, 0:2, :])
        nc.sync.dma_start(out=x16[:, b, 2:4, :], in_=xv[:, 2:4, :])
        nc.scalar.dma_start(out=x16[:, b, 4:6, :], in_=xv[:, 4:6, :])
        nc.scalar.dma_start(out=x16[0:32, b, 6, :], in_=x_d[b, 768:800, :])
    nc.scalar.dma_start(out=muf, in_=muf_d)

    # ---------------- stages ----------------
    la = [
        ps_la.tile([128, NCH, K], f32, tag="la", name=f"la{b}")
        for b in range(B_LOC)
    ]
    xt_ps = {}
    pool_ps = {}

    def bias(b):
        # One whole-tile bias matmul opens the bank's accumulation: a
        # start=True write marks the ENTIRE psum bank pending-zero, so
        # there must be exactly one start per bank, covering everything
        # the chunk matmuls later accumulate into.
        nb_bc = nbo[0:2, 0:64].unsqueeze(1).broadcast_to((2, NCH, K))
        nc.tensor.matmul(
            la[b],
            lhsT=nbo[0:2, 64:192],
            rhs=nb_bc,
            start=True, stop=False, skip_group_check=True,
        )

    def tx(b, g):
        c0, ncc, toff, _ = GROUPS[g]
        p = ps_xt.tile([128, 2, 512], f16, tag="xt", name=f"xt{b}{g}")
        xt_ps[(b, g)] = p
        for h in range(2):
            for c in range(c0, c0 + ncc):
                t0, tcn = CHUNKS[c]
                nc.tensor.transpose(
                    p[:, h, t0 - toff : t0 - toff + tcn],
                    x16[0:tcn, b, c, h * 128 : (h + 1) * 128],
                    ident[0:tcn, 0:tcn],
                )

    def evac(b, g):
        # one wide copy per stream, alternating ACT/DVE across streams
        _, _, toff, tw = GROUPS[g]
        p = xt_ps[(b, g)]
        dst = xT[:, b, :, toff : toff + tw]
        nc.scalar.copy(dst, p[:, :, 0:tw])

    def mm1(b, g):
        c0, ncc, _, _ = GROUPS[g]
        for c in range(c0, c0 + ncc):
            t0, tcn = CHUNKS[c]
            for h in range(2):
                nc.tensor.matmul(
                    la[b][0:tcn, c, :],
                    lhsT=xT[:, b, h, t0 : t0 + tcn],
                    rhs=musT[h],
                    start=False,
                    stop=(c == c0 + ncc - 1 and h == 1),
                    skip_group_check=True,
                )

    def smf(b, g):
        # softmax front: row max (negated), subtract, exp
        c0, ncc, _, _ = GROUPS[g]
        cs = slice(c0, c0 + ncc)
        nc.vector.tensor_reduce(
            out=nm[:, b, cs], in_=la[b][:, cs, :],
            axis=mybir.AxisListType.X, op=ALU.max, negate=True,
        )
        nm_bc = nm[:, b, cs].unsqueeze(2).broadcast_to((128, ncc, K))
        nc.vector.tensor_add(s[:, b, cs, :], la[b][:, cs, :], nm_bc)
        nc.scalar.activation(out=e[:, b, cs, :], in_=s[:, b, cs, :], func=AF.Exp)

    def smb(b, g):
        # softmax back: Z, 2^15/Z, r~ = e * (2^15/Z)
        c0, ncc, _, _ = GROUPS[g]
        cs = slice(c0, c0 + ncc)
        nc.vector.tensor_reduce(
            out=z[:, b, cs], in_=e[:, b, cs, :],
            axis=mybir.AxisListType.X, op=ALU.add,
        )
        nc.vector.reciprocal(zr[:, b, cs], z[:, b, cs])
        # scale r~ by 2^15 before the fp16 cast so near-dead components'
        # tiny weights stay above the fp16 flush threshold; the epilogue
        # divides by (S + 2^15*eps), which cancels the scale exactly.
        nc.vector.tensor_scalar_mul(zr[:, b, cs], zr[:, b, cs], 32768.0)
        zr_bc = zr[:, b, cs].unsqueeze(2).broadcast_to((128, ncc, K))
        nc.gpsimd.tensor_mul(r16[:, b, cs, :], e[:, b, cs, :], zr_bc)

    def mm2(b, g):
        c0, ncc, _, _ = GROUPS[g]
        if g == 0:
            pool_ps[b] = ps_p.tile([K, D + 1], f32, tag="pool", name=f"pool{b}")
        for c in range(c0, c0 + ncc):
            t0, tcn = CHUNKS[c]
            nc.tensor.matmul(
                pool_ps[b],
                lhsT=r16[0:tcn, b, c, :],
                rhs=x16[0:tcn, b, c, 0 : D + 1],
                start=(c == 0),
                stop=(c == NCH - 1),
            )

    def epilogue(b):
        # po = (pool - mu*S) * Sr,  Sr = 1/(S + 2^15*eps); exactly
        # pool*Sr - mu*(S*Sr) but with a 3-hop chain instead of 5.
        pp = pool_ps[b]
        negS = epip.tile([K, 1], f32, tag="negS")
        se = epip.tile([K, 1], f32, tag="se")
        sr = epip.tile([K, 1], f32, tag="sr")
        q = epip.tile([K, D], f32, tag="q")
        po = epip.tile([K, D], f32, tag="po")
        S = pp[:, D : D + 1]
        nc.vector.tensor_scalar_mul(negS, S, -1.0)
        nc.vector.tensor_scalar_add(se, S, EPS * 32768.0)
        nc.vector.reciprocal(sr, se)
        nc.vector.scalar_tensor_tensor(
            out=q, in0=muf, scalar=negS, in1=pp[:, 0:D],
            op0=ALU.mult, op1=ALU.add,
        )
        nc.vector.tensor_scalar_mul(po, q, sr)
        nc.sync.dma_start(
            out=out_d[b, :].rearrange("(k d) -> k d", k=K), in_=po
        )

    # ---------------- software-pipelined emission ----------------
    S0, S1, S2, S3 = STREAMS
    tx(*S0)
    tx(*S1)
    bias(0)
    evac(*S0)
    mm1(*S0)
    tx(*S2)
    bias(1)
    evac(*S1)
    mm1(*S1)
    evac(*S2)
    smf(*S0)
    tx(*S3)
    mm1(*S2)
    evac(*S3)
    smf(*S1)
    smb(*S0)
    mm1(*S3)
    smf(*S2)
    smb(*S1)
    mm2(*S0)
    smf(*S3)
    smb(*S2)
    mm2(*S1)
    mm2(*S2)
    smb(*S3)
    mm2(*S3)
    epilogue(0)
    epilogue(1)
    ctx.close()


_NC = None


def _get_nc():
    global _NC
    if _NC is None:
        import concourse.bacc as bacc
        import concourse.tile as tile
        from concourse import mybir

        f32 = mybir.dt.float32
        f16 = mybir.dt.float16
        nc = bacc.Bacc(
            "TRN2", target_bir_lowering=False, debug=False, num_devices=N_CORES
        )
        x_d = nc.dram_tensor(
            "x16", [B_LOC, T, DP], f16, kind="ExternalInput"
        ).ap()
        c16a_d = nc.dram_tensor(
            "c16a", [128, 256], f16, kind="ExternalInput"
        ).ap()
        nbo_d = nc.dram_tensor("nbo", [2, 192], f16, kind="ExternalInput").ap()
        muf_d = nc.dram_tensor("muf", [K, D], f32, kind="ExternalInput").ap()
        out_d = nc.dram_tensor(
            "out", [B_LOC, K * D], f32, kind="ExternalOutput"
        ).ap()
        with tile.TileContext(nc) as tc:
            _emit(tc, x_d, c16a_d, nbo_d, muf_d, out_d)
        nc.compile()
        _NC = nc
    return _NC


def _host_prep(x, mu, prec):
    """Host-side input prep: fp16 x with ones col, const tensors."""
    x = np.asarray(x, dtype=np.float32)
    mu64 = np.asarray(mu, dtype=np.float64)
    p2 = np.asarray(prec, dtype=np.float64) ** 2

    x16 = np.zeros((B, T, DP), dtype=np.float16)
    x16[:, :, 0:D] = x.astype(np.float16)
    x16[:, :, D] = 1.0

    mu_s16 = (2.0 * p2[:, None] * mu64).astype(np.float16)  # [K, D]
    c16a = np.zeros((128, 256), dtype=np.float16)
    c16a[:, 0:128] = np.eye(128, dtype=np.float16)
    c16a[:, 128:192] = mu_s16.T[0:128, :]
    c16a[:, 192:256] = mu_s16.T[128:256, :]

    nb = -(p2 * (mu64 * mu64).sum(-1))  # [K], exact in f64
    nb_hi = nb.astype(np.float16)
    nb_lo = (nb - nb_hi.astype(np.float64)).astype(np.float16)
    nbo = np.zeros((2, 192), dtype=np.float16)
    nbo[0, 0:64] = nb_hi
    nbo[1, 0:64] = nb_lo
    nbo[:, 64:192] = 1.0

    muf = np.ascontiguousarray(mu64.astype(np.float32))
    return x16, c16a, nbo, muf


def kernel(x, mu, prec, **_ignored):
    from concourse.bass_utils import run_bass_kernel_spmd

    x16, c16a, nbo, muf = _host_prep(x, mu, prec)
    nc = _get_nc()
    in_maps = [
        {
            "x16": np.ascontiguousarray(x16[c * B_LOC : (c + 1) * B_LOC]),
            "c16a": c16a,
            "nbo": nbo,
            "muf": muf,
        }
        for c in range(N_CORES)
    ]
    res = run_bass_kernel_spmd(nc, in_maps, list(range(N_CORES)))
    return np.concatenate(
        [res.results[c]["out"] for c in range(N_CORES)], axis=0
    ).astype(np.float32)
